# revision 72
# baseline (speedup 1.0000x reference)
"""Multi-head self-attention (RoPE, causal) Trainium2 Bass kernel.

Full inputs in, full output out. Sharding: 8 cores = 2 batch x 4 head-groups
(4 heads each). Software-pipelined iteration i emits
  - qkv projection + RoPE + layout-permute for token-quarter tq=i,
  - output projection + store for query-chunk pc=i-2,
  - streaming causal attention for query-chunk qc=i-1 (high priority so the
    Tile list-scheduler favors its exp chain; qkv/proj fill PE gaps).
Attention operands (q, k, v, exp(scores), rope math) are bf16; projections
f32r. Causal mask applied on the PE via an accumulated -1e9 upper-triangular
matmul; off-diagonal score tiles exp'd in merged 1024-col pairs. Softmax
denominator via a ones-column in v-hat, reciprocal broadcast on gpsimd.
Host sums the 4 per-batch partials and adds the (bv@Wproj + bproj) constant.

Self-contained: hardcodes all shapes for B=2, T=2048, D=1024, H=16, hd=64.
"""
from contextlib import ExitStack

import numpy as np
import ml_dtypes

from concourse import bacc, mybir, tile
from concourse.bass_utils import run_bass_kernel_spmd

f32 = mybir.dt.float32
f32r = mybir.dt.float32r
bf16 = mybir.dt.bfloat16
fp8 = mybir.dt.float8e4
DR = mybir.MatmulPerfMode.DoubleRow
EXP = mybir.ActivationFunctionType.Exp
IDENT = mybir.ActivationFunctionType.Identity

B, T, D = 2, 2048, 1024
H, HD = 16, 64
HALF = HD // 2  # 32
HPC = 4  # heads per core
BASE = 10000.0
NTQ = 4  # token quarters of 512
NQC = 4  # query chunks of 512
NKT = 16  # key tiles of 128


def _build():
    nc = bacc.Bacc("TRN2", target_bir_lowering=False, debug=False, num_devices=8)

    xT = nc.dram_tensor("xT", [D, T], bf16, kind="ExternalInput").ap()
    wqk = nc.dram_tensor("wqk", [D, 512], bf16, kind="ExternalInput").ap()
    wv = nc.dram_tensor("wv", [D, 256], bf16, kind="ExternalInput").ap()
    wp = nc.dram_tensor("wp", [256, D], bf16, kind="ExternalInput").ap()
    bqk = nc.dram_tensor("bqk", [128, 4], f32, kind="ExternalInput").ap()
    cosb = nc.dram_tensor("cosb", [128, T], bf16, kind="ExternalInput").ap()
    sinb = nc.dram_tensor("sinb", [128, T], bf16, kind="ExternalInput").ap()
    maskT_d = nc.dram_tensor("maskT_d", [128, 128], bf16, kind="ExternalInput").ap()
    ident_d = nc.dram_tensor("ident_d", [128, 128], bf16, kind="ExternalInput").ap()
    outT = nc.dram_tensor("outT", [D, T], f32, kind="ExternalOutput").ap()

    with tile.TileContext(nc) as tc, ExitStack() as ctx:
        consts = ctx.enter_context(tc.tile_pool(name="consts", bufs=1, side="right"))
        wpool = ctx.enter_context(tc.tile_pool(name="wpool", bufs=1, side="right"))
        xt_pool = ctx.enter_context(tc.tile_pool(name="xt", bufs=2))
        eo_pool = ctx.enter_context(tc.tile_pool(name="eo", bufs=4))
        tmp_pool = ctx.enter_context(tc.tile_pool(name="tmp", bufs=4))
        fs_pool = ctx.enter_context(tc.tile_pool(name="fs", bufs=4))
        qr_pool = ctx.enter_context(tc.tile_pool(name="qr", bufs=1, side="right"))
        vh_pool = ctx.enter_context(tc.tile_pool(name="vh", bufs=1, side="right"))
        at_pool = ctx.enter_context(tc.tile_pool(name="at", bufs=6))
        small = ctx.enter_context(tc.tile_pool(name="small", bufs=4))
        rb_pool = ctx.enter_context(tc.tile_pool(name="rb", bufs=4))
        ot_pool = ctx.enter_context(tc.tile_pool(name="ot", bufs=1, side="right"))
        ob_pool = ctx.enter_context(tc.tile_pool(name="ob", bufs=2))
        ps_qk = ctx.enter_context(tc.tile_pool(name="ps_qk", bufs=2, space="PSUM"))
        ps_s = ctx.enter_context(tc.tile_pool(name="ps_s", bufs=2, space="PSUM"))
        ps_o = ctx.enter_context(tc.tile_pool(name="ps_o", bufs=2, space="PSUM"))

        # ---- persistent SBUF tensors ----
        wqk_t = wpool.tile([128, 8, 512], bf16, tag="wqk_t")
        wv_t = wpool.tile([128, 8, 256], bf16, tag="wv_t")
        wp_t = wpool.tile([128, 2, D], bf16, tag="wp_t")
        cos_t = consts.tile([128, T], bf16, tag="cos_t")
        sin_t = consts.tile([128, T], bf16, tag="sin_t")
        maskT_t = consts.tile([128, 128], bf16, tag="maskT_t")
        ident_t = consts.tile([128, 128], bf16, tag="ident_t")
        bqk_t = consts.tile([128, 4], f32, tag="bqk_t")
        # rotated q/k in DoubleRow form: partition 32h+theta, free (deck,t)
        qr2 = qr_pool.tile([128, 2, T], fp8, tag="qr2")
        kr2 = qr_pool.tile([128, 2, T], fp8, tag="kr2")
        # v-hat: [key-in-tile, kt, h, 64 v-dims + ones col]
        vhat = vh_pool.tile([128, NKT, HPC, HD + 1], bf16, tag="vhat")
        otA = ot_pool.tile([128, T], bf16, tag="otA")
        otB = ot_pool.tile([128, T], bf16, tag="otB")

        # ---- constant loads on the scalar HWDGE queue (wqk first so the
        # ---- first matmuls can start); x0 per-chunk on sync; ones column of
        # ---- vhat via memset on gpsimd.
        wqkr = wqk.rearrange("(dn p) c -> p dn c", dn=8)
        nc.scalar.dma_start(wqk_t[:, 0:1, :], wqkr[:, 0:1, :])
        nc.scalar.dma_start(wqk_t[:, 1:3, :], wqkr[:, 1:3, :])
        nc.scalar.dma_start(wqk_t[:, 3:5, :], wqkr[:, 3:5, :])
        nc.scalar.dma_start(wqk_t[:, 5:8, :], wqkr[:, 5:8, :])
        nc.scalar.dma_start(bqk_t[:], bqk)
        nc.gpsimd.memset(vhat[:, :, :, HD:HD + 1], 1.0)
        # pin lifetimes of mid-program-first-written persistents so the SBUF
        # allocator cannot overlap them with released early tiles
        for tns in (qr2, kr2, otA, otB):
            nc.gpsimd.memset(tns[:], 0.0)

        xts = {}

        def load_x(tq):
            xt = xt_pool.tile([128, 8, 512], bf16, tag="xt", name=f"xt{tq}")
            t0 = tq * 512
            if tq == 0:
                xr = xT.rearrange("(dn p) t -> p dn t", dn=8)
                for hf in range(4):
                    nc.sync.dma_start(
                        xt[:, 2 * hf:2 * hf + 2, :],
                        xr[:, 2 * hf:2 * hf + 2, t0:t0 + 512])
            else:
                nc.sync.dma_start(
                    xt[:],
                    xT.rearrange("(dn p) t -> p dn t", dn=8)[:, :, t0:t0 + 512])
            xts[tq] = xt

        def emit_qkv(tq):
            """qkv projection + rope + permute for token quarter tq."""
            xc = xts.pop(tq)
            t0 = tq * 512
            sl = slice(t0, t0 + 512)
            eos = []
            for ch in range(4):
                pqk = ps_qk.tile([128, 512], f32, tag="qk",
                                 name=f"pqk{tq}_{ch}")
                for dn in range(8):
                    nc.tensor.matmul(
                        pqk[:],
                        wqk_t[:, dn, ch * 128:(ch + 1) * 128],
                        xc[:, dn, :],
                        start=(dn == 0), stop=(dn == 7))
                eo = eo_pool.tile([128, 512], bf16, tag="eo",
                                  name=f"eo{tq}_{ch}")
                nc.scalar.activation(eo[:], pqk[:], IDENT,
                                     bias=bqk_t[:, ch:ch + 1], scale=1.0)
                eos.append(eo)
                if ch % 2 == 1:
                    # rope for q (ch pair 0,1) or k (2,3): bf16 math on DVE,
                    # fp8 output straight into the DoubleRow-form tensors
                    qk = ch // 2
                    E, O = eos[2 * qk], eos[2 * qk + 1]
                    dst = qr2 if qk == 0 else kr2
                    t1 = tmp_pool.tile([128, 512], bf16, tag="tmp")
                    t2 = tmp_pool.tile([128, 512], bf16, tag="tmp")
                    nc.vector.tensor_mul(t1[:], E[:], cos_t[:, sl])
                    nc.vector.tensor_mul(t2[:], O[:], sin_t[:, sl])
                    nc.vector.tensor_sub(dst[:, 0, sl], t1[:], t2[:])
                    t3 = tmp_pool.tile([128, 512], bf16, tag="tmp")
                    t4 = tmp_pool.tile([128, 512], bf16, tag="tmp")
                    nc.vector.tensor_mul(t3[:], E[:], sin_t[:, sl])
                    nc.vector.tensor_mul(t4[:], O[:], cos_t[:, sl])
                    nc.vector.tensor_add(dst[:, 1, sl], t3[:], t4[:])
            for tt in range(4):
                pv = ps_qk.tile([128, 256], f32, tag="qk",
                                name=f"pv{tq}_{tt}")
                for dn in range(8):
                    nc.tensor.matmul(
                        pv[:],
                        xc[:, dn, tt * 128:(tt + 1) * 128],
                        wv_t[:, dn, :],
                        start=(dn == 0), stop=(dn == 7))
                nc.vector.tensor_copy(
                    vhat[:, tq * 4 + tt, :, 0:HD],
                    pv[:].rearrange("p (h c) -> p h c", h=HPC))

        def emit_proj(pc):
            """output projection + store for query chunk pc."""
            q0 = pc * 512
            ob = ob_pool.tile([128, 8, 512], f32, tag="ob", name=f"ob{pc}")
            for oc in range(8):
                pool = ps_s if (pc == 3 and oc % 2) else ps_qk
                tag = "s" if (pc == 3 and oc % 2) else "qk"
                pj = pool.tile([128, 512], f32, tag=tag,
                               name=f"pj{pc}_{oc}")
                order = ((1, otB), (0, otA)) if pc == 3 else ((0, otA), (1, otB))
                for step, (hd, ot) in enumerate(order):
                    nc.tensor.matmul(pj[:], wp_t[:, hd, oc * 128:(oc + 1) * 128],
                                     ot[:, q0:q0 + 512],
                                     start=(step == 0), stop=(step == 1))
                if oc % 2 or pc < 3:
                    nc.vector.tensor_copy(ob[:, oc, :], pj[:])
                else:
                    nc.scalar.copy(ob[:, oc, :], pj[:])
            nsp = 8 if pc == 3 else 2
            w = 8 // nsp
            for hf in range(nsp):
                nc.sync.dma_start(
                    outT.rearrange("(oc p) t -> p oc t", oc=8)[
                        :, w * hf:w * hf + w, q0:q0 + 512],
                    ob[:, w * hf:w * hf + w, :])

        def emit_norm(qc, h, po):
            # normalize: recip of denom row, broadcast on gpsimd
            q0 = qc * 512
            recip = small.tile([1, 512], f32, tag="recip",
                               name=f"recip{qc}_{h}")
            nc.vector.reciprocal(recip[:], po[HD:HD + 1, :])
            rb = rb_pool.tile([64, 512], f32, tag="rb",
                              name=f"rb{qc}_{h}")
            nc.gpsimd.partition_broadcast(rb[:], recip[:])
            ot = otA if h < 2 else otB
            r0 = (h % 2) * 64
            nc.vector.tensor_mul(ot[r0:r0 + 64, q0:q0 + 512],
                                 po[0:HD, :], rb[:])

        def emit_attention(qc):
            """streaming causal attention for query chunk qc. The AV matmul
            lags the exp by one group ACROSS head boundaries so the PE never
            drains waiting for a head's last exp."""
            q0 = qc * 512
            nkt = 4 * qc + 4
            # drain chunk: heads feeding otB first so the projection's otB
            # half can start while heads 0/1 still run
            horder = (2, 3, 0, 1) if qc == NQC - 1 else range(HPC)
            pend = None  # (h, po, group, at, is_last_of_head)
            for h in horder:
                p0 = 32 * h
                po = ps_o.tile([HD + 1, 512], f32, tag="o",
                               name=f"po{qc}_{h}")
                groups = [(kt, kt + 1) for kt in range(0, 4 * qc, 2)]
                groups += [(kt,) for kt in range(4 * qc, nkt)]
                for gi, g in enumerate(groups):
                    w = 512 * len(g)
                    pss = ps_s.tile([128, w], f32, tag="s",
                                    name=f"pss{qc}_{h}_{g[0]}")
                    at = at_pool.tile([128, w], bf16, tag="at",
                                      name=f"at{qc}_{h}_{g[0]}")
                    col_lo = 0
                    for j, kt in enumerate(g):
                        k0 = kt * 128
                        col_lo = k0 - q0 if k0 > q0 else 0
                        diag = kt >= 4 * qc
                        nc.tensor.matmul(
                            pss[:, 512 * j + col_lo:512 * (j + 1)],
                            kr2[p0:p0 + 32, :, k0:k0 + 128],
                            qr2[p0:p0 + 32, :, q0 + col_lo:q0 + 512],
                            start=True, stop=True, perf_mode=DR,
                            tile_position=(p0, 0))
                        if diag:  # causal mask: accumulate -1e9 upper tri
                            nc.tensor.matmul(
                                pss[:, 512 * j + col_lo:
                                    512 * j + col_lo + 128],
                                maskT_t[:], ident_t[:],
                                start=False, stop=True,
                                skip_group_check=True)
                    # one exp over the whole group (col_lo only != 0 for
                    # single-tile diagonal groups)
                    nc.scalar.activation(at[:, col_lo:], pss[:, col_lo:], EXP)
                    if pend is not None:
                        ph, ppo, pg, pat, plast = pend
                        emit_av(qc, ph, ppo, pg, pat)
                        if plast:
                            emit_norm(qc, ph, ppo)
                    pend = (h, po, g, at, gi == len(groups) - 1)
            ph, ppo, pg, pat, plast = pend
            emit_av(qc, ph, ppo, pg, pat)
            emit_norm(qc, ph, ppo)

        def emit_av(qc, h, po, g, at):
            q0 = qc * 512
            nkt = 4 * qc + 4
            for j, kt in enumerate(g):
                k0 = kt * 128
                col_lo = k0 - q0 if k0 > q0 else 0
                nc.tensor.matmul(
                    po[:, col_lo:512],
                    vhat[:, kt, h, :],
                    at[:, 512 * j + col_lo:512 * (j + 1)],
                    start=(kt == 0), stop=(kt == nkt - 1),
                    skip_group_check=True)

        load_x(0)
        nc.sync.dma_start(cos_t[:], cosb)
        nc.sync.dma_start(sin_t[:], sinb)
        nc.sync.dma_start(wv_t[:], wv.rearrange("(dn p) c -> p dn c", dn=8))
        load_x(1)
        nc.sync.dma_start(maskT_t[:], maskT_d)
        nc.sync.dma_start(ident_t[:], ident_d)
        nc.sync.dma_start(wp_t[:], wp.rearrange("(hd p) c -> p hd c", hd=2))

        for it in range(NTQ + 2):
            tq = it if it < NTQ else None
            qc = it - 1 if 1 <= it <= NTQ else None
            pc = it - 2 if it >= 2 else None

            if tq is not None:
                emit_qkv(tq)
            if pc is not None:
                emit_proj(pc)
            if qc is not None:
                with tc.high_priority(offset=100000):
                    emit_attention(qc)
            if tq is not None and tq + 2 < NTQ:
                load_x(tq + 2)

    nc.compile()
    return nc


_NC = None


def _get_nc():
    global _NC
    if _NC is None:
        _NC = _build()
    return _NC


def _host_prep(x, Wqkv, bqkv, Wproj, bproj, pos):
    """Build the 8 per-core input maps."""
    x = np.asarray(x, dtype=np.float32)
    Wqkv = np.asarray(Wqkv, dtype=np.float32)
    bqkv = np.asarray(bqkv, dtype=np.float32)
    Wproj = np.asarray(Wproj, dtype=np.float32)
    bproj = np.asarray(bproj, dtype=np.float32)
    pos = int(np.asarray(pos))

    scale = HD ** -0.5
    # rope tables, layout [128 = 4 heads x 32 thetas (h-major), T]
    theta = 1.0 / BASE ** (np.arange(HALF, dtype=np.float32) / HALF)
    angles = np.outer(np.arange(pos, pos + T, dtype=np.float32), theta)  # [T,32]
    cosT = np.cos(angles).T.astype(np.float32)  # [32, T]
    sinT = np.sin(angles).T.astype(np.float32)
    cos4 = np.ascontiguousarray(np.tile(cosT, (4, 1)))  # [128, T]
    sin4 = np.ascontiguousarray(np.tile(sinT, (4, 1)))

    # additive causal mask, fed through PE: psum[k, j] += maskT[j, k], with
    # maskT[a, b] = -1e9 iff b > a (strictly-upper triangle)
    maskT = np.triu(np.full((128, 128), -1e9, dtype=np.float32), k=1)
    maskT = np.ascontiguousarray(maskT).astype(ml_dtypes.bfloat16)
    ident = np.eye(128, dtype=np.float32).astype(ml_dtypes.bfloat16)

    in_maps = []
    for c in range(8):
        b, hg = c // 4, c % 4
        heads = [4 * hg + h for h in range(HPC)]
        permE = np.array([h * HD + 2 * i for h in heads for i in range(HALF)])
        permO = permE + 1
        wqk_np = np.concatenate([
            Wqkv[:, permE] * scale,          # qE
            Wqkv[:, permO] * scale,          # qO
            Wqkv[:, D + permE],              # kE
            Wqkv[:, D + permO],              # kO
        ], axis=1)
        bqk_np = np.stack([
            bqkv[permE] * scale, bqkv[permO] * scale,
            bqkv[D + permE], bqkv[D + permO],
        ], axis=1)
        wv_np = Wqkv[:, 2 * D + 256 * hg: 2 * D + 256 * (hg + 1)]
        wp_np = Wproj[256 * hg: 256 * (hg + 1), :]
        in_maps.append({
            "xT": np.ascontiguousarray(x[b].T).astype(ml_dtypes.bfloat16),
            "wqk": np.ascontiguousarray(wqk_np).astype(ml_dtypes.bfloat16),
            "wv": np.ascontiguousarray(wv_np).astype(ml_dtypes.bfloat16),
            "wp": np.ascontiguousarray(wp_np).astype(ml_dtypes.bfloat16),
            "bqk": np.ascontiguousarray(bqk_np, dtype=np.float32),
            "cosb": cos4.astype(ml_dtypes.bfloat16),
            "sinb": sin4.astype(ml_dtypes.bfloat16),
            "maskT_d": maskT,
            "ident_d": ident,
        })
    const_vec = bqkv[2 * D:] @ Wproj + bproj  # exact host-side bias handling
    return in_maps, const_vec


def kernel(x, Wqkv, bqkv, Wproj, bproj, pos, **kw):
    in_maps, const_vec = _host_prep(x, Wqkv, bqkv, Wproj, bproj, pos)
    nc = _get_nc()
    res = run_bass_kernel_spmd(nc, in_maps, core_ids=list(range(8))).results
    out = np.empty((B, T, D), dtype=np.float32)
    for b in range(B):
        acc = res[4 * b]["outT"].copy()
        for c in range(4 * b + 1, 4 * b + 4):
            acc += res[c]["outT"]
        out[b] = acc.T + const_vec
    return out


# revision 73
# speedup vs baseline: 1.0046x; 1.0046x over previous
"""Multi-head self-attention (RoPE, causal) Trainium2 Bass kernel.

Full inputs in, full output out. Sharding: 8 cores = 2 batch x 4 head-groups
(4 heads each). Software-pipelined iteration i emits
  - qkv projection + RoPE + layout-permute for token-quarter tq=i,
  - output projection + store for query-chunk pc=i-2,
  - streaming causal attention for query-chunk qc=i-1 (high priority so the
    Tile list-scheduler favors its exp chain; qkv/proj fill PE gaps).
Attention operands (q, k, v, exp(scores), rope math) are bf16; projections
f32r. Causal mask applied on the PE via an accumulated -1e9 upper-triangular
matmul; off-diagonal score tiles exp'd in merged 1024-col pairs. Softmax
denominator via a ones-column in v-hat, reciprocal broadcast on gpsimd.
Host sums the 4 per-batch partials and adds the (bv@Wproj + bproj) constant.

Self-contained: hardcodes all shapes for B=2, T=2048, D=1024, H=16, hd=64.
"""
from contextlib import ExitStack

import numpy as np
import ml_dtypes

from concourse import bacc, mybir, tile
from concourse.bass_utils import run_bass_kernel_spmd

f32 = mybir.dt.float32
f32r = mybir.dt.float32r
bf16 = mybir.dt.bfloat16
fp8 = mybir.dt.float8e4
DR = mybir.MatmulPerfMode.DoubleRow
EXP = mybir.ActivationFunctionType.Exp
IDENT = mybir.ActivationFunctionType.Identity

B, T, D = 2, 2048, 1024
H, HD = 16, 64
HALF = HD // 2  # 32
HPC = 4  # heads per core
BASE = 10000.0
NTQ = 4  # token quarters of 512
NQC = 4  # query chunks of 512
NKT = 16  # key tiles of 128


def _build():
    nc = bacc.Bacc("TRN2", target_bir_lowering=False, debug=False, num_devices=8)

    xT = nc.dram_tensor("xT", [D, T], bf16, kind="ExternalInput").ap()
    wqk = nc.dram_tensor("wqk", [D, 512], bf16, kind="ExternalInput").ap()
    wv = nc.dram_tensor("wv", [D, 256], bf16, kind="ExternalInput").ap()
    wp = nc.dram_tensor("wp", [256, D], bf16, kind="ExternalInput").ap()
    bqk = nc.dram_tensor("bqk", [128, 4], f32, kind="ExternalInput").ap()
    cosb = nc.dram_tensor("cosb", [128, T], bf16, kind="ExternalInput").ap()
    sinb = nc.dram_tensor("sinb", [128, T], bf16, kind="ExternalInput").ap()
    maskT_d = nc.dram_tensor("maskT_d", [128, 128], bf16, kind="ExternalInput").ap()
    ident_d = nc.dram_tensor("ident_d", [128, 128], bf16, kind="ExternalInput").ap()
    outT = nc.dram_tensor("outT", [D, T], f32, kind="ExternalOutput").ap()

    with tile.TileContext(nc) as tc, ExitStack() as ctx:
        consts = ctx.enter_context(tc.tile_pool(name="consts", bufs=1, side="right"))
        wpool = ctx.enter_context(tc.tile_pool(name="wpool", bufs=1, side="right"))
        xt_pool = ctx.enter_context(tc.tile_pool(name="xt", bufs=2))
        eo_pool = ctx.enter_context(tc.tile_pool(name="eo", bufs=4))
        tmp_pool = ctx.enter_context(tc.tile_pool(name="tmp", bufs=4))
        fs_pool = ctx.enter_context(tc.tile_pool(name="fs", bufs=4))
        qr_pool = ctx.enter_context(tc.tile_pool(name="qr", bufs=1, side="right"))
        vh_pool = ctx.enter_context(tc.tile_pool(name="vh", bufs=1, side="right"))
        at_pool = ctx.enter_context(tc.tile_pool(name="at", bufs=6))
        small = ctx.enter_context(tc.tile_pool(name="small", bufs=4))
        rb_pool = ctx.enter_context(tc.tile_pool(name="rb", bufs=4))
        ot_pool = ctx.enter_context(tc.tile_pool(name="ot", bufs=1, side="right"))
        ob_pool = ctx.enter_context(tc.tile_pool(name="ob", bufs=2))
        ps_qk = ctx.enter_context(tc.tile_pool(name="ps_qk", bufs=2, space="PSUM"))
        ps_s = ctx.enter_context(tc.tile_pool(name="ps_s", bufs=2, space="PSUM"))
        ps_o = ctx.enter_context(tc.tile_pool(name="ps_o", bufs=2, space="PSUM"))

        # ---- persistent SBUF tensors ----
        wqk_t = wpool.tile([128, 8, 512], bf16, tag="wqk_t")
        wv_t = wpool.tile([128, 8, 256], bf16, tag="wv_t")
        wp_t = wpool.tile([128, 2, D], bf16, tag="wp_t")
        cos_t = consts.tile([128, T], bf16, tag="cos_t")
        sin_t = consts.tile([128, T], bf16, tag="sin_t")
        maskT_t = consts.tile([128, 128], bf16, tag="maskT_t")
        ident_t = consts.tile([128, 128], bf16, tag="ident_t")
        bqk_t = consts.tile([128, 4], f32, tag="bqk_t")
        # rotated q/k in DoubleRow form: partition 32h+theta, free (deck,t)
        qr2 = qr_pool.tile([128, 2, T], fp8, tag="qr2")
        kr2 = qr_pool.tile([128, 2, T], fp8, tag="kr2")
        # v-hat: [key-in-tile, kt, h, 64 v-dims + ones col]
        vhat = vh_pool.tile([128, NKT, HPC, HD + 1], bf16, tag="vhat")
        otA = ot_pool.tile([128, T], bf16, tag="otA")
        otB = ot_pool.tile([128, T], bf16, tag="otB")

        # ---- constant loads on the scalar HWDGE queue (wqk first so the
        # ---- first matmuls can start); x0 per-chunk on sync; ones column of
        # ---- vhat via memset on gpsimd.
        wqkr = wqk.rearrange("(dn p) c -> p dn c", dn=8)
        nc.scalar.dma_start(wqk_t[:, 0:1, :], wqkr[:, 0:1, :])
        nc.scalar.dma_start(wqk_t[:, 1:3, :], wqkr[:, 1:3, :])
        nc.scalar.dma_start(wqk_t[:, 3:5, :], wqkr[:, 3:5, :])
        nc.scalar.dma_start(wqk_t[:, 5:8, :], wqkr[:, 5:8, :])
        nc.scalar.dma_start(bqk_t[:], bqk)
        nc.gpsimd.memset(vhat[:, :, :, HD:HD + 1], 1.0)
        # pin lifetimes of mid-program-first-written persistents so the SBUF
        # allocator cannot overlap them with released early tiles
        for tns in (qr2, kr2, otA, otB):
            nc.gpsimd.memset(tns[:], 0.0)

        xts = {}

        def load_x(tq):
            xt = xt_pool.tile([128, 8, 512], bf16, tag="xt", name=f"xt{tq}")
            t0 = tq * 512
            if tq == 0:
                xr = xT.rearrange("(dn p) t -> p dn t", dn=8)
                for hf in range(4):
                    nc.sync.dma_start(
                        xt[:, 2 * hf:2 * hf + 2, :],
                        xr[:, 2 * hf:2 * hf + 2, t0:t0 + 512])
            else:
                nc.sync.dma_start(
                    xt[:],
                    xT.rearrange("(dn p) t -> p dn t", dn=8)[:, :, t0:t0 + 512])
            xts[tq] = xt

        def emit_qkv(tq):
            """qkv projection + rope + permute for token quarter tq."""
            xc = xts.pop(tq)
            t0 = tq * 512
            sl = slice(t0, t0 + 512)
            eos = []
            for ch in range(4):
                pqk = ps_qk.tile([128, 512], f32, tag="qk",
                                 name=f"pqk{tq}_{ch}")
                for dn in range(8):
                    nc.tensor.matmul(
                        pqk[:],
                        wqk_t[:, dn, ch * 128:(ch + 1) * 128],
                        xc[:, dn, :],
                        start=(dn == 0), stop=(dn == 7))
                eo = eo_pool.tile([128, 512], bf16, tag="eo",
                                  name=f"eo{tq}_{ch}")
                nc.scalar.activation(eo[:], pqk[:], IDENT,
                                     bias=bqk_t[:, ch:ch + 1], scale=1.0)
                eos.append(eo)
                if ch % 2 == 1:
                    # rope for q (ch pair 0,1) or k (2,3): bf16 math on DVE,
                    # fp8 output straight into the DoubleRow-form tensors
                    qk = ch // 2
                    E, O = eos[2 * qk], eos[2 * qk + 1]
                    dst = qr2 if qk == 0 else kr2
                    with tc.high_priority(offset=50000):
                        t1 = tmp_pool.tile([128, 512], bf16, tag="tmp")
                        t2 = tmp_pool.tile([128, 512], bf16, tag="tmp")
                        nc.vector.tensor_mul(t1[:], E[:], cos_t[:, sl])
                        nc.vector.tensor_mul(t2[:], O[:], sin_t[:, sl])
                        nc.vector.tensor_sub(dst[:, 0, sl], t1[:], t2[:])
                        t3 = tmp_pool.tile([128, 512], bf16, tag="tmp")
                        t4 = tmp_pool.tile([128, 512], bf16, tag="tmp")
                        nc.vector.tensor_mul(t3[:], E[:], sin_t[:, sl])
                        nc.vector.tensor_mul(t4[:], O[:], cos_t[:, sl])
                        nc.vector.tensor_add(dst[:, 1, sl], t3[:], t4[:])
            for tt in range(4):
                pv = ps_qk.tile([128, 256], f32, tag="qk",
                                name=f"pv{tq}_{tt}")
                for dn in range(8):
                    nc.tensor.matmul(
                        pv[:],
                        xc[:, dn, tt * 128:(tt + 1) * 128],
                        wv_t[:, dn, :],
                        start=(dn == 0), stop=(dn == 7))
                nc.vector.tensor_copy(
                    vhat[:, tq * 4 + tt, :, 0:HD],
                    pv[:].rearrange("p (h c) -> p h c", h=HPC))

        def emit_proj(pc):
            """output projection + store for query chunk pc."""
            q0 = pc * 512
            ob = ob_pool.tile([128, 8, 512], f32, tag="ob", name=f"ob{pc}")
            for oc in range(8):
                pool = ps_s if (pc == 3 and oc % 2) else ps_qk
                tag = "s" if (pc == 3 and oc % 2) else "qk"
                pj = pool.tile([128, 512], f32, tag=tag,
                               name=f"pj{pc}_{oc}")
                order = ((1, otB), (0, otA)) if pc == 3 else ((0, otA), (1, otB))
                for step, (hd, ot) in enumerate(order):
                    nc.tensor.matmul(pj[:], wp_t[:, hd, oc * 128:(oc + 1) * 128],
                                     ot[:, q0:q0 + 512],
                                     start=(step == 0), stop=(step == 1))
                if oc % 2 or pc < 3:
                    nc.vector.tensor_copy(ob[:, oc, :], pj[:])
                else:
                    nc.scalar.copy(ob[:, oc, :], pj[:])
            nsp = 8 if pc == 3 else 2
            w = 8 // nsp
            for hf in range(nsp):
                nc.sync.dma_start(
                    outT.rearrange("(oc p) t -> p oc t", oc=8)[
                        :, w * hf:w * hf + w, q0:q0 + 512],
                    ob[:, w * hf:w * hf + w, :])

        def emit_norm(qc, h, po):
            # normalize: recip of denom row, broadcast on gpsimd
            q0 = qc * 512
            recip = small.tile([1, 512], f32, tag="recip",
                               name=f"recip{qc}_{h}")
            nc.vector.reciprocal(recip[:], po[HD:HD + 1, :])
            rb = rb_pool.tile([64, 512], f32, tag="rb",
                              name=f"rb{qc}_{h}")
            nc.gpsimd.partition_broadcast(rb[:], recip[:])
            ot = otA if h < 2 else otB
            r0 = (h % 2) * 64
            nc.vector.tensor_mul(ot[r0:r0 + 64, q0:q0 + 512],
                                 po[0:HD, :], rb[:])

        def emit_attention(qc):
            """streaming causal attention for query chunk qc. The AV matmul
            lags the exp by one group ACROSS head boundaries so the PE never
            drains waiting for a head's last exp."""
            q0 = qc * 512
            nkt = 4 * qc + 4
            # drain chunk: heads feeding otB first so the projection's otB
            # half can start while heads 0/1 still run
            horder = (2, 3, 0, 1) if qc == NQC - 1 else range(HPC)
            pend = None  # (h, po, group, at, is_last_of_head)
            for h in horder:
                p0 = 32 * h
                po = ps_o.tile([HD + 1, 512], f32, tag="o",
                               name=f"po{qc}_{h}")
                groups = [(kt, kt + 1) for kt in range(0, 4 * qc, 2)]
                groups += [(kt,) for kt in range(4 * qc, nkt)]
                for gi, g in enumerate(groups):
                    w = 512 * len(g)
                    pss = ps_s.tile([128, w], f32, tag="s",
                                    name=f"pss{qc}_{h}_{g[0]}")
                    at = at_pool.tile([128, w], bf16, tag="at",
                                      name=f"at{qc}_{h}_{g[0]}")
                    col_lo = 0
                    for j, kt in enumerate(g):
                        k0 = kt * 128
                        col_lo = k0 - q0 if k0 > q0 else 0
                        diag = kt >= 4 * qc
                        nc.tensor.matmul(
                            pss[:, 512 * j + col_lo:512 * (j + 1)],
                            kr2[p0:p0 + 32, :, k0:k0 + 128],
                            qr2[p0:p0 + 32, :, q0 + col_lo:q0 + 512],
                            start=True, stop=True, perf_mode=DR,
                            tile_position=(p0, 0))
                        if diag:  # causal mask: accumulate -1e9 upper tri
                            nc.tensor.matmul(
                                pss[:, 512 * j + col_lo:
                                    512 * j + col_lo + 128],
                                maskT_t[:], ident_t[:],
                                start=False, stop=True,
                                skip_group_check=True)
                    # one exp over the whole group (col_lo only != 0 for
                    # single-tile diagonal groups)
                    nc.scalar.activation(at[:, col_lo:], pss[:, col_lo:], EXP)
                    if pend is not None:
                        ph, ppo, pg, pat, plast = pend
                        emit_av(qc, ph, ppo, pg, pat)
                        if plast:
                            emit_norm(qc, ph, ppo)
                    pend = (h, po, g, at, gi == len(groups) - 1)
            ph, ppo, pg, pat, plast = pend
            emit_av(qc, ph, ppo, pg, pat)
            emit_norm(qc, ph, ppo)

        def emit_av(qc, h, po, g, at):
            q0 = qc * 512
            nkt = 4 * qc + 4
            for j, kt in enumerate(g):
                k0 = kt * 128
                col_lo = k0 - q0 if k0 > q0 else 0
                nc.tensor.matmul(
                    po[:, col_lo:512],
                    vhat[:, kt, h, :],
                    at[:, 512 * j + col_lo:512 * (j + 1)],
                    start=(kt == 0), stop=(kt == nkt - 1),
                    skip_group_check=True)

        load_x(0)
        nc.sync.dma_start(cos_t[:], cosb)
        nc.sync.dma_start(sin_t[:], sinb)
        nc.sync.dma_start(wv_t[:], wv.rearrange("(dn p) c -> p dn c", dn=8))
        load_x(1)
        nc.sync.dma_start(maskT_t[:], maskT_d)
        nc.sync.dma_start(ident_t[:], ident_d)
        nc.sync.dma_start(wp_t[:], wp.rearrange("(hd p) c -> p hd c", hd=2))

        for it in range(NTQ + 2):
            tq = it if it < NTQ else None
            qc = it - 1 if 1 <= it <= NTQ else None
            pc = it - 2 if it >= 2 else None

            if tq is not None:
                emit_qkv(tq)
            if pc is not None:
                emit_proj(pc)
            if qc is not None:
                with tc.high_priority(offset=100000):
                    emit_attention(qc)
            if tq is not None and tq + 2 < NTQ:
                load_x(tq + 2)

    nc.compile()
    return nc


_NC = None


def _get_nc():
    global _NC
    if _NC is None:
        _NC = _build()
    return _NC


def _host_prep(x, Wqkv, bqkv, Wproj, bproj, pos):
    """Build the 8 per-core input maps."""
    x = np.asarray(x, dtype=np.float32)
    Wqkv = np.asarray(Wqkv, dtype=np.float32)
    bqkv = np.asarray(bqkv, dtype=np.float32)
    Wproj = np.asarray(Wproj, dtype=np.float32)
    bproj = np.asarray(bproj, dtype=np.float32)
    pos = int(np.asarray(pos))

    scale = HD ** -0.5
    # rope tables, layout [128 = 4 heads x 32 thetas (h-major), T]
    theta = 1.0 / BASE ** (np.arange(HALF, dtype=np.float32) / HALF)
    angles = np.outer(np.arange(pos, pos + T, dtype=np.float32), theta)  # [T,32]
    cosT = np.cos(angles).T.astype(np.float32)  # [32, T]
    sinT = np.sin(angles).T.astype(np.float32)
    cos4 = np.ascontiguousarray(np.tile(cosT, (4, 1)))  # [128, T]
    sin4 = np.ascontiguousarray(np.tile(sinT, (4, 1)))

    # additive causal mask, fed through PE: psum[k, j] += maskT[j, k], with
    # maskT[a, b] = -1e9 iff b > a (strictly-upper triangle)
    maskT = np.triu(np.full((128, 128), -1e9, dtype=np.float32), k=1)
    maskT = np.ascontiguousarray(maskT).astype(ml_dtypes.bfloat16)
    ident = np.eye(128, dtype=np.float32).astype(ml_dtypes.bfloat16)

    in_maps = []
    for c in range(8):
        b, hg = c // 4, c % 4
        heads = [4 * hg + h for h in range(HPC)]
        permE = np.array([h * HD + 2 * i for h in heads for i in range(HALF)])
        permO = permE + 1
        wqk_np = np.concatenate([
            Wqkv[:, permE] * scale,          # qE
            Wqkv[:, permO] * scale,          # qO
            Wqkv[:, D + permE],              # kE
            Wqkv[:, D + permO],              # kO
        ], axis=1)
        bqk_np = np.stack([
            bqkv[permE] * scale, bqkv[permO] * scale,
            bqkv[D + permE], bqkv[D + permO],
        ], axis=1)
        wv_np = Wqkv[:, 2 * D + 256 * hg: 2 * D + 256 * (hg + 1)]
        wp_np = Wproj[256 * hg: 256 * (hg + 1), :]
        in_maps.append({
            "xT": np.ascontiguousarray(x[b].T).astype(ml_dtypes.bfloat16),
            "wqk": np.ascontiguousarray(wqk_np).astype(ml_dtypes.bfloat16),
            "wv": np.ascontiguousarray(wv_np).astype(ml_dtypes.bfloat16),
            "wp": np.ascontiguousarray(wp_np).astype(ml_dtypes.bfloat16),
            "bqk": np.ascontiguousarray(bqk_np, dtype=np.float32),
            "cosb": cos4.astype(ml_dtypes.bfloat16),
            "sinb": sin4.astype(ml_dtypes.bfloat16),
            "maskT_d": maskT,
            "ident_d": ident,
        })
    const_vec = bqkv[2 * D:] @ Wproj + bproj  # exact host-side bias handling
    return in_maps, const_vec


def kernel(x, Wqkv, bqkv, Wproj, bproj, pos, **kw):
    in_maps, const_vec = _host_prep(x, Wqkv, bqkv, Wproj, bproj, pos)
    nc = _get_nc()
    res = run_bass_kernel_spmd(nc, in_maps, core_ids=list(range(8))).results
    out = np.empty((B, T, D), dtype=np.float32)
    for b in range(B):
        acc = res[4 * b]["outT"].copy()
        for c in range(4 * b + 1, 4 * b + 4):
            acc += res[c]["outT"]
        out[b] = acc.T + const_vec
    return out
